# revision 23
# baseline (speedup 1.0000x reference)
"""Trainium2 Bass kernel for nn_CrossAttention (B=2, N=2048, C=1024, H=16, D=64).

Sharding: 8 cores = 2 batches x 4 head-groups (4 heads each).
Each core computes its head-group's attention + a partial output projection;
the host sums the 4 partials per batch and adds the bias.

Device pipeline per core (v2 - engine-balanced, phase-interleaved):
  All matmul operands bf16 (f32r measured ~1.7x slower per row on HW).
  KV phase: project+norm+rope all 16 context chunks. Variance via ACT Square
    (ACT idle here), PSUM evacs on ACT, DVE does reduce/rstd/rope (rstd applied
    via one broadcast-view tensor_tensor).
  Q chunks + gate for q-block 0, then per q-block qc: attention pair loops
    (paired score matmuls -> ACT exp from 2-bank PSUM, scale=1/8, no max
    subtraction - rms-normed q,k bound |score| <= 8 -> paired attn@v + M=1
    ones matmuls for denominators), with the Q/gate projections for qc+1
    INTERLEAVED into the score loops so the exp-bound attention phase and the
    DVE/PE-bound projection phase overlap. Q-phase evacs/square go to DVE
    (ACT is the bottleneck during attention). Gating + out-proj evacs on DVE.
  PSUM budget (8 banks): scores 2x[128,1024] (4) + acc ao/dn/outproj
    rotation 2x[128,512] (2) + proj [128,512] (1) + transpose [128,256] (1).
"""

import os
import sys
import numpy as np

for _p in ("/opt/trn_rl_repo", "/opt/pypackages"):
    if _p not in sys.path:
        sys.path.insert(0, _p)

B, N, C = 2, 2048, 1024
H, D = 16, 64
HG = 4            # heads per core
NCH = 16          # token chunks of 128
QB = 4            # q blocks of 512
KTC = 16          # key chunks of 128
EPS = 1e-6

_PROG = None      # cached compiled Bass program
LAST_EXEC_NS = None
LAST_PROFILE = None


def _build_program():
    import concourse.bass as bass
    import concourse.bacc as bacc
    import concourse.tile as tile
    import concourse.mybir as mybir

    F32 = mybir.dt.float32
    BF = mybir.dt.bfloat16
    AF = mybir.ActivationFunctionType
    OP = mybir.AluOpType

    nc = bacc.Bacc("TRN2", target_bir_lowering=False, debug=False, num_devices=8)

    xT = nc.dram_tensor("xT", [8, 128, N], BF, kind="ExternalInput")
    ctxT = nc.dram_tensor("ctxT", [8, 128, N], BF, kind="ExternalInput")
    wq = nc.dram_tensor("wq", [8, 128, 256], BF, kind="ExternalInput")
    wg = nc.dram_tensor("wg", [8, 128, 256], BF, kind="ExternalInput")
    wkv = nc.dram_tensor("wkv", [8, 128, 512], BF, kind="ExternalInput")
    wo = nc.dram_tensor("wo", [2, 128, 1024], BF, kind="ExternalInput")
    cosq = nc.dram_tensor("cosq", [N, D], BF, kind="ExternalInput")
    ssinq = nc.dram_tensor("ssinq", [N, D], BF, kind="ExternalInput")
    cosk = nc.dram_tensor("cosk", [N, D], BF, kind="ExternalInput")
    ssink = nc.dram_tensor("ssink", [N, D], BF, kind="ExternalInput")
    part = nc.dram_tensor("part", [N, C], F32, kind="ExternalOutput")

    def bcast4(ap):
        # [128, 64] -> [128, 4, 64] with step-0 middle dim (read-broadcast)
        return bass.AP(tensor=ap.tensor, offset=ap.offset,
                       ap=[ap.ap[0], [0, 4], ap.ap[1]])

    def bcast_rstd(ap):
        # [128, 4] -> [128, 64, 4] d-major view broadcasting each head's
        # scalar over d (keeps the zero stride out of the innermost dim)
        return bass.AP(tensor=ap.tensor, offset=ap.offset,
                       ap=[ap.ap[0], [0, 64], ap.ap[1]])

    def dmajor(ap):
        # [128, 4, 64] -> [128, 64, 4] transposed free-dim view
        p, hdim, ddim = ap.ap
        return bass.AP(tensor=ap.tensor, offset=ap.offset,
                       ap=[p, ddim, hdim])

    def swap_view(ap):
        # ap: [128, 4, 64] contiguous -> per head read order d+32..d+63, d..d+31
        p, hdim, ddim = ap.ap
        return bass.AP(tensor=ap.tensor, offset=ap.offset + 32 * ddim[0],
                       ap=[p, hdim, [-32 * ddim[0], 2], [ddim[0], 32]])

    with tile.TileContext(nc) as tc:
        import contextlib
        with contextlib.ExitStack() as ctx:
            singles = ctx.enter_context(tc.tile_pool(name="singles", bufs=1))
            slices = ctx.enter_context(tc.tile_pool(name="slices", bufs=3))
            work = ctx.enter_context(tc.tile_pool(name="work", bufs=2))
            persist = ctx.enter_context(tc.tile_pool(name="persist", bufs=1))
            exps_p = ctx.enter_context(tc.tile_pool(name="exps", bufs=6))
            gat_p = ctx.enter_context(tc.tile_pool(name="gat", bufs=2))
            pools = {}  # phase-dependent PSUM pools: 'proj' and 'tp'

            # ---- first-needed weights/tables (DMA order matters) ----
            wkv_sb = singles.tile([128, 8, 512], BF)
            nc.sync.dma_start(out=wkv_sb, in_=wkv.ap().rearrange("c p f -> p c f"))
            ck_sb = singles.tile([128, NCH, D], BF)
            nc.sync.dma_start(out=ck_sb, in_=cosk.ap().rearrange("(i p) d -> p i d", p=128))
            sk_sb = singles.tile([128, NCH, D], BF)
            nc.sync.dma_start(out=sk_sb, in_=ssink.ap().rearrange("(i p) d -> p i d", p=128))
            from concourse.masks import make_identity
            ident = singles.tile([128, 128], BF)
            make_identity(nc, ident)
            ones1 = singles.tile([128, 1], BF)
            nc.vector.memset(ones1, 1.0)
            ones2 = singles.tile([128, 64], BF)
            nc.vector.memset(ones2, 1.0)
            eps_sb = singles.tile([128, 1], F32)
            nc.vector.memset(eps_sb, EPS)

            # ---- persistent intermediates ----
            pairK = persist.tile([128, 2, N], BF, tag="pairK")
            pairQb = [persist.tile([128, 2, 512], BF, tag=f"pairQ{q}",
                                   name=f"pairQ{q}") for q in range(QB)]
            v_sb = persist.tile([128, KTC, 4, 64], BF, tag="v_sb")
            graw = [persist.tile([128, 2, 512], BF, tag=f"graw{q}",
                                 name=f"graw{q}") for q in range(QB)]
            A_sb = [persist.tile([128, 2, 512], BF, tag=f"A{q}",
                                 name=f"A{q}") for q in range(QB)]

            def qk_stages(mode, sl, ns, j, w_sb, wcols, cos_t, sin_t, dst,
                          dslice):
                """Staged project+norm+rope+transpose for chunk j of q
                (mode='q') or k/v. Returns [s0..s4]: s0-s2 issue 2-3 proj MMs
                each, s3 the last MMs plus the ACT/DVE norm+rope chain, s4
                (schedule >=2 steps later) the PE transposes + evac so the PE
                queue never waits on the cross-engine chain.

                mode 'kv' (ACT idle) -> evacs on ACT; 'q' (attention-bound
                ACT) -> evacs on DVE.
                """
                box = {}

                def mms(c0, c1):
                    for c in range(c0, c1):
                        nc.tensor.matmul(box["ps"][:, :wcols],
                                         sl[:, c, ns * 128:(ns + 1) * 128],
                                         w_sb[:, c, :],
                                         start=(c == 0), stop=(c == 7))

                def s0():
                    box["ps"] = pools["proj"].tile([128, 512], F32,
                                                   tag="proj", name="ps")
                    mms(0, 2)

                def s3():
                    mms(6, 8)
                    ps = box["ps"]
                    qpart = ps[:, 0:256]
                    qhd = qpart.rearrange("p (h d) -> p h d", h=4)
                    # variance (zero-mean folded into host-centered weights)
                    sqv = work.tile([128, 256], F32, tag="sq")
                    nc.scalar.activation(out=sqv, in_=qpart, func=AF.Square)
                    ssum = work.tile([128, 4], F32, tag="ssum")
                    nc.vector.tensor_reduce(
                        out=ssum, in_=sqv.rearrange("p (h d) -> p h d", h=4),
                        axis=mybir.AxisListType.X, op=OP.add)
                    sdev = work.tile([128, 4], F32, tag="sdev")
                    nc.scalar.activation(out=sdev, in_=ssum, func=AF.Sqrt,
                                         scale=1.0 / 64.0, bias=eps_sb)
                    rstd = work.tile([128, 4], F32, tag="rstd")
                    nc.vector.reciprocal(out=rstd, in_=sdev)
                    qn = work.tile([128, 4, 64], BF, tag="qn")
                    nc.vector.tensor_tensor(out=dmajor(qn), in0=dmajor(qhd),
                                            in1=bcast_rstd(rstd), op=OP.mult)
                    # rope: qr = qn*cos + swap(qn)*ssin (sign folded in ssin)
                    t1 = work.tile([128, 4, 64], BF, tag="t1")
                    nc.vector.tensor_tensor(out=t1, in0=qn, in1=bcast4(cos_t),
                                            op=OP.mult)
                    t2 = work.tile([128, 4, 64], BF, tag="t2")
                    nc.vector.tensor_tensor(out=t2, in0=swap_view(qn),
                                            in1=bcast4(sin_t), op=OP.mult)
                    qr = work.tile([128, 4, 64], BF, tag="qr")
                    nc.vector.tensor_tensor(out=qr, in0=t1, in1=t2, op=OP.add)
                    box["qr"] = qr
                    if mode == "kv":
                        # v evac on ACT
                        nc.scalar.activation(
                            out=v_sb[:, j, :, :],
                            in_=ps[:, 256:512].rearrange("p (h d) -> p h d",
                                                         h=4),
                            func=AF.Copy)

                def s4():
                    # PE transposes: both head-pairs into ONE psum bank
                    # (T1 start clears the bank; T2 must not re-clear)
                    qr = box["qr"]
                    pst = pools["mk_tp"]()
                    for p in range(2):
                        nc.tensor.matmul(
                            pst[:, p * 128:(p + 1) * 128],
                            qr[:, 2 * p:2 * p + 2, :]
                            .rearrange("p a b -> p (a b)"),
                            ident, is_transpose=True,
                            start=(p == 0), stop=(p == 1),
                            skip_group_check=True)
                    dst_ap = dst[:, :, dslice]
                    src_ap = pst.rearrange("p (a b) -> p a b", a=2)
                    if mode == "kv":
                        nc.scalar.activation(out=dst_ap, in_=src_ap,
                                             func=AF.Copy)
                    else:
                        nc.vector.tensor_copy(out=dst_ap, in_=src_ap)

                return [s0, lambda: mms(2, 4), lambda: mms(4, 6), s3, s4]

            def kv_stages(c_sl, ns, j):
                return qk_stages("kv", c_sl, ns, j, wkv_sb, 512,
                                 ck_sb[:, j, :], sk_sb[:, j, :],
                                 pairK, slice(j * 128, (j + 1) * 128))

            def q_stages(x_sl, ns, qcn):
                j = qcn * 4 + ns
                return qk_stages("q", x_sl, ns, j, wq_sb, 256,
                                 cq_sb[:, j, :], sq_sb[:, j, :],
                                 pairQb[qcn], slice(ns * 128, (ns + 1) * 128))

            def gate_stages(x_sl, gfc, qcn):
                """Gate projection split into 4 hook steps of 2 MMs."""
                box = {}

                def mms(c0, c1):
                    if "psg" not in box:
                        box["psg"] = pools["proj"].tile([128, 512], F32,
                                                        tag="proj",
                                                        name="psg")
                    for c in range(c0, c1):
                        nc.tensor.matmul(
                            box["psg"], wg_sb[:, c, gfc * 128:(gfc + 1) * 128],
                            x_sl[:, c, :], start=(c == 0), stop=(c == 7))

                def last():
                    mms(6, 8)
                    nc.vector.tensor_copy(out=graw[qcn][:, gfc, :],
                                          in_=box["psg"])

                return [lambda: mms(0, 2), lambda: mms(2, 4),
                        lambda: mms(4, 6), last]

            # ================= phase A: KV + Q block 0 =================
            wq_sb = singles.tile([128, 8, 256], BF)
            wg_sb = singles.tile([128, 8, 256], BF)
            wo_sb = singles.tile([128, 2, 1024], BF)
            cq_sb = singles.tile([128, NCH, D], BF)
            sq_sb = singles.tile([128, NCH, D], BF)

            def load_x_slice(qcn):
                x_sl = slices.tile([128, 8, 512], BF, tag="slice", name="x_sl")
                nc.sync.dma_start(
                    out=x_sl,
                    in_=xT.ap()[:, :, qcn * 512:(qcn + 1) * 512]
                    .rearrange("c p n -> p c n"))
                return x_sl

            with tc.tile_pool(name="psP", bufs=4, space="PSUM") as psP, \
                 tc.tile_pool(name="psTT", bufs=2, space="PSUM") as psTT:
                pools["proj"] = psP
                pools["mk_tp"] = lambda: psTT.tile([128, 256], BF, tag="tp",
                                                   name="pst")
                prev_s4 = None
                for qc4 in range(4):
                    c_sl = slices.tile([128, 8, 512], BF, tag="slice", name="c_sl")
                    nc.sync.dma_start(
                        out=c_sl,
                        in_=ctxT.ap()[:, :, qc4 * 512:(qc4 + 1) * 512]
                        .rearrange("c p n -> p c n"))
                    if qc4 == 0:
                        # q-side weights/tables behind the first ctx slice
                        nc.sync.dma_start(out=wq_sb, in_=wq.ap().rearrange("c p f -> p c f"))
                        nc.sync.dma_start(out=wg_sb, in_=wg.ap().rearrange("c p f -> p c f"))
                        nc.sync.dma_start(out=cq_sb, in_=cosq.ap().rearrange("(i p) d -> p i d", p=128))
                        nc.sync.dma_start(out=sq_sb, in_=ssinq.ap().rearrange("(i p) d -> p i d", p=128))
                        nc.sync.dma_start(out=wo_sb, in_=wo.ap().rearrange("c p f -> p c f"))
                    for ns in range(4):
                        st = kv_stages(c_sl, ns, qc4 * 4 + ns)
                        for s in st[:4]:
                            s()
                        # previous chunk's transposes after this chunk's MMs:
                        # by then its norm/rope chain has drained, so the PE
                        # never head-of-line blocks on DVE
                        if prev_s4 is not None:
                            prev_s4()
                        prev_s4 = st[4]

                x_sl0 = load_x_slice(0)
                for ns in range(4):
                    st = q_stages(x_sl0, ns, 0)
                    for s in st[:4]:
                        s()
                    if prev_s4 is not None:
                        prev_s4()
                    prev_s4 = st[4]
                for gfc in range(2):
                    for s in gate_stages(x_sl0, gfc, 0):
                        s()
                prev_s4()

            # ================= attention + interleaved next-block proj =======
            # Fully software-pipelined flat loop over (qc, pair, k-chunk):
            # per step: flush oldest pending attn@v/denominator MMs, then
            # score MMs + exp, then hooks (gating / out-proj / next-block
            # projections) so exp never waits behind bunched boundary work.
            psSC = ctx.enter_context(tc.tile_pool(name="psSC", bufs=2, space="PSUM"))
            psACC = ctx.enter_context(tc.tile_pool(name="psACC", bufs=3, space="PSUM"))
            psA = ctx.enter_context(tc.tile_pool(name="psA", bufs=1, space="PSUM"))
            pools["proj"] = psA
            # interleaved-phase transposes reuse the current step's
            # just-consumed scores bank (no extra alloc: ring parity and the
            # exp pipeline are preserved; WAR on the exp read is tracked)
            cur_sc = {}
            pools["mk_tp"] = lambda: cur_sc["ps"].bitcast(BF)[:, 0:256]

            state = {}   # (qc, p) -> (ao, dn) PSUM tiles

            def flush_attn(eS, qc, p, k):
                if (qc, p) not in state:
                    # dn before ao: ring slot rotation then ties each new
                    # alloc to the oldest pair's earliest-finishing readers
                    dn_p = psACC.tile([128, 512], F32, tag="acc", name="dn")
                    ao_p = psACC.tile([128, 512], F32, tag="acc", name="ao")
                    state[(qc, p)] = (ao_p, dn_p)
                ao_p, dn_p = state[(qc, p)]
                # first MM of the k==0 group clears the whole bank;
                # the second must NOT re-clear (would drop the
                # first's has_written bits) -> start only on MM1.
                st = (k == 0)
                sp = (k == KTC - 1)
                nc.tensor.matmul(ao_p[0:64, :], v_sb[:, k, 2 * p, :],
                                 eS[:, 0, :], start=st, stop=sp,
                                 tile_position=(0, 0),
                                 skip_group_check=True)
                nc.tensor.matmul(ao_p[64:128, :], v_sb[:, k, 2 * p + 1, :],
                                 eS[:, 1, :], start=st, stop=sp,
                                 tile_position=(0, 64),
                                 skip_group_check=True)
                # denominator rows 0 and 64: opposite column quadrants so the
                # two M=1 streams run concurrently
                nc.tensor.matmul(dn_p[0:1, :], ones1, eS[:, 0, :],
                                 start=st, stop=sp,
                                 tile_position=(0, 0),
                                 skip_group_check=True)
                nc.tensor.matmul(dn_p[64:65, :], ones1, eS[:, 1, :],
                                 start=st, stop=sp,
                                 tile_position=(0, 64),
                                 skip_group_check=True)

            def gating(qc, p):
                # (DVE; reciprocal-denominator broadcast via PE ones matmul)
                ao_p, dn_p = state.pop((qc, p))
                gs = gat_p.tile([128, 512], F32, tag="gs")
                nc.scalar.activation(out=gs, in_=graw[qc][:, p, :],
                                     func=AF.Tanh, scale=0.5)
                sig = gat_p.tile([128, 512], F32, tag="sig")
                nc.vector.tensor_scalar(out=sig, in0=gs, scalar1=0.5,
                                        scalar2=0.5, op0=OP.mult,
                                        op1=OP.add)
                dns = gat_p.tile([128, 512], BF, tag="dns")
                nc.vector.tensor_copy(out=dns[0:1, :], in_=dn_p[0:1, :])
                nc.vector.tensor_copy(out=dns[64:65, :], in_=dn_p[64:65, :])
                rbc = pools["proj"].tile([128, 512], F32, tag="proj",
                                         name="rbc")
                nc.tensor.matmul(rbc[0:64, :], ones2[0:1, :], dns[0:1, :],
                                 start=True, stop=True,
                                 tile_position=(0, 0))
                nc.tensor.matmul(rbc[64:128, :], ones2[64:65, :],
                                 dns[64:65, :], start=True, stop=True,
                                 tile_position=(64, 64))
                rec = gat_p.tile([128, 512], F32, tag="rec")
                nc.vector.reciprocal(out=rec, in_=rbc)
                m = gat_p.tile([128, 512], F32, tag="m")
                nc.vector.tensor_tensor(out=m, in0=sig, in1=rec, op=OP.mult)
                nc.vector.tensor_tensor(out=A_sb[qc][:, p, :], in0=ao_p,
                                        in1=m, op=OP.mult)

            def outproj_piece(qc, i):
                nk, oc = divmod(i, 2)
                n1 = qc * 4 + nk
                po = pools["proj"].tile([128, 512], F32, tag="proj", name="po")
                for fc in range(2):
                    nc.tensor.matmul(
                        po,
                        A_sb[qc][:, fc, nk * 128:(nk + 1) * 128],
                        wo_sb[:, fc, oc * 512:(oc + 1) * 512],
                        start=(fc == 0), stop=(fc == 1))
                ev = gat_p.tile([128, 512], F32, tag="ev")
                nc.vector.tensor_copy(out=ev, in_=po)
                nc.sync.dma_start(
                    out=part.ap()[n1 * 128:(n1 + 1) * 128,
                                  oc * 512:(oc + 1) * 512],
                    in_=ev)

            hooks = {}

            def add_hook(qc, p, k, fn):
                hooks.setdefault((qc, p, k), []).append(fn)

            def at(qc, p, k):
                # normalize a possibly-overflowing (qc, p, k) step address
                t = qc * 32 + p * 16 + k
                return t // 32, (t % 32) // 16, t % 16

            def add_stages(first, stages, gap=1):
                """Register stage list at consecutive steps (gap apart); the
                final stage (transpose+evac) runs 3 steps after the prior."""
                qc, p, k = first
                for i, s in enumerate(stages[:-1]):
                    add_hook(*at(qc, p, k + i * gap), s)
                add_hook(*at(qc, p, k + (len(stages) - 2) * gap + 3),
                         stages[-1])

            for qc in range(4):
                # gating as soon as the pair's accumulation completes
                # (flushes trail the score loop by 3 steps)
                add_hook(qc, 1, 2, lambda qc=qc: gating(qc, 0))
                if qc > 0:
                    add_hook(qc, 0, 2, lambda qc=qc: gating(qc - 1, 1))
                    for i in range(8):
                        add_hook(qc, 0, 3 + i,
                                 lambda qc=qc, i=i: outproj_piece(qc - 1, i))
                if qc + 1 < QB:
                    # next q-block projections spread through this window
                    qcn = qc + 1
                    b = []
                    add_hook(qc, 0, 1, lambda qcn=qcn, b=b:
                             b.append(load_x_slice(qcn)))

                    def lazy_stages(factory, n):
                        box2 = {}

                        def run(i):
                            if "s" not in box2:
                                box2["s"] = factory()
                            box2["s"][i]()

                        return [lambda i=i: run(i) for i in range(n)]

                    def mk(ns, qcn=qcn, b=b):
                        return lazy_stages(
                            lambda: q_stages(b[0], ns, qcn), 5)

                    add_stages((qc, 0, 11), mk(0))
                    add_stages((qc, 0, 15), mk(1))
                    add_stages((qc, 1, 3), mk(2))
                    add_stages((qc, 1, 7), mk(3))

                    def mkg(gfc, qcn=qcn, b=b):
                        return lazy_stages(
                            lambda: gate_stages(b[0], gfc, qcn), 4)

                    g0 = mkg(0)
                    g1 = mkg(1)
                    for i in range(4):
                        add_hook(*at(qc, 1, 11 + i), g0[i])
                        add_hook(*at(qc, 1, 15 + i), g1[i])

            pend = []  # (expS tile, qc, p, ktc) awaiting attn MMs
            for qc in range(4):
                for p in range(2):
                    for k in range(KTC):
                        if len(pend) >= 3:
                            flush_attn(*pend.pop(0))
                        ksl = slice(k * 128, (k + 1) * 128)
                        ps = psSC.tile([128, 1024], F32, tag="sc", name="sc")
                        nc.tensor.matmul(ps[:, 0:512],
                                         pairK[0:64, p, ksl],
                                         pairQb[qc][0:64, p, :],
                                         start=True, stop=True,
                                         tile_position=(0, 0))
                        nc.tensor.matmul(ps[:, 512:1024],
                                         pairK[64:128, p, ksl],
                                         pairQb[qc][64:128, p, :],
                                         start=True, stop=True,
                                         tile_position=(64, 0))
                        eS = exps_p.tile([128, 2, 512], BF, tag="expS",
                                         name="eS")
                        nc.scalar.activation(
                            out=eS.rearrange("p a b -> p (a b)"), in_=ps,
                            func=AF.Exp, scale=0.125)
                        pend.append((eS, qc, p, k))
                        cur_sc["ps"] = ps
                        for fn in hooks.pop((qc, p, k), ()):
                            fn()
            for e in pend:
                flush_attn(*e)
            gating(3, 1)
            for i in range(8):
                outproj_piece(3, i)

    nc.compile()
    return nc


def _prep_core(inputs, b, g, bf16):
    x = np.asarray(inputs["x"][b], dtype=np.float32)
    ctx = np.asarray(inputs["context"][b], dtype=np.float32)
    Wq = np.asarray(inputs["Wq"], dtype=np.float32).reshape(H, 2 * D, C)
    Wkv = np.asarray(inputs["Wkv"], dtype=np.float32).reshape(H, 2 * D, C)
    Wo = np.asarray(inputs["Wo"], dtype=np.float32)
    cos = np.asarray(inputs["cos"][b], dtype=np.float32)
    sin = np.asarray(inputs["sin"][b], dtype=np.float32)
    qw = np.asarray(inputs["q_norm_w"], dtype=np.float32)
    kw = np.asarray(inputs["k_norm_w"], dtype=np.float32)

    hs = slice(HG * g, HG * g + HG)
    qr = Wq[hs, :D, :]                       # [4, D, C]
    qr = qr - qr.mean(axis=1, keepdims=True)
    gr = Wq[hs, D:, :]
    kr = Wkv[hs, :D, :]
    kr = kr - kr.mean(axis=1, keepdims=True)
    vr = Wkv[hs, D:, :]

    sgn = np.where(np.arange(D) < D // 2, -1.0, 1.0).astype(np.float32)
    wswap = lambda w: np.concatenate([w[D // 2:], w[:D // 2]])

    return {
        "xT": np.ascontiguousarray(x.T).reshape(8, 128, N).astype(bf16),
        "ctxT": np.ascontiguousarray(ctx.T).reshape(8, 128, N).astype(bf16),
        "wq": np.ascontiguousarray(
            qr.reshape(HG * D, C).T).reshape(8, 128, 256).astype(bf16),
        "wg": np.ascontiguousarray(
            gr.reshape(HG * D, C).T).reshape(8, 128, 256).astype(bf16),
        "wkv": np.ascontiguousarray(
            np.concatenate([kr.reshape(HG * D, C), vr.reshape(HG * D, C)], 0).T
        ).reshape(8, 128, 512).astype(bf16),
        "wo": np.ascontiguousarray(
            Wo[:, 256 * g:256 * (g + 1)].T).reshape(2, 128, C).astype(bf16),
        "cosq": (cos * qw[None, :]).astype(bf16),
        "ssinq": (sin * sgn[None, :] * wswap(qw)[None, :]).astype(bf16),
        "cosk": (cos * kw[None, :]).astype(bf16),
        "ssink": (sin * sgn[None, :] * wswap(kw)[None, :]).astype(bf16),
    }


def kernel(**inputs):
    global _PROG, LAST_EXEC_NS, LAST_PROFILE
    import ml_dtypes
    bf16 = ml_dtypes.bfloat16

    if _PROG is None:
        _PROG = _build_program()
    nc = _PROG

    in_maps = [_prep_core(inputs, core // 4, core % 4, bf16) for core in range(8)]

    trace = bool(os.environ.get("BASS_KERNEL_TRACE"))
    kw = {}
    if trace:
        import types
        from trn_agent_boot.trn_boot import _ntff_profile_via_ctypes
        hook = _ntff_profile_via_ctypes('/opt/axon/libaxon_pjrt.so')
        mod = types.ModuleType('antenv.axon_hooks')
        mod.get_axon_ntff_profile_hook = lambda: hook
        sys.modules['antenv.axon_hooks'] = mod
        from concourse import bass_utils
        bass_utils.upload_artifacts = lambda tmpdir: tmpdir
        kw = dict(trace=True, tmpdir=os.environ.get("BASS_KERNEL_TRACE_DIR"))

    from concourse.bass_utils import run_bass_kernel_spmd
    res = run_bass_kernel_spmd(nc, in_maps, core_ids=list(range(8)), **kw)
    LAST_EXEC_NS = res.exec_time_ns
    LAST_PROFILE = res.profile_json

    bo = np.asarray(inputs["bo"], dtype=np.float32)
    out = np.zeros((B, N, C), dtype=np.float32)
    for core in range(8):
        out[core // 4] += res.results[core]["part"]
    out += bo[None, None, :]
    return out


# revision 28
# speedup vs baseline: 1.2565x; 1.2565x over previous
"""Trainium2 Bass kernel for nn_CrossAttention (B=2, N=2048, C=1024, H=16, D=64).

Sharding: 8 cores = 2 batches x 4 head-groups (4 heads each).
Each core computes its head-group's attention + a partial output projection;
the host sums the 4 partials per batch and adds the bias.

Device pipeline per core (v2 - engine-balanced, phase-interleaved):
  All matmul operands bf16 (f32r measured ~1.7x slower per row on HW).
  KV phase: project+norm+rope all 16 context chunks. Variance via ACT Square
    (ACT idle here), PSUM evacs on ACT, DVE does reduce/rstd/rope (rstd applied
    via one broadcast-view tensor_tensor).
  Q chunks + gate for q-block 0, then per q-block qc: attention pair loops
    (paired score matmuls -> ACT exp from 2-bank PSUM, scale=1/8, no max
    subtraction - rms-normed q,k bound |score| <= 8 -> paired attn@v + M=1
    ones matmuls for denominators), with the Q/gate projections for qc+1
    INTERLEAVED into the score loops so the exp-bound attention phase and the
    DVE/PE-bound projection phase overlap. Q-phase evacs/square go to DVE
    (ACT is the bottleneck during attention). Gating + out-proj evacs on DVE.
  PSUM budget (8 banks): scores 2x[128,1024] (4) + acc ao/dn/outproj
    rotation 2x[128,512] (2) + proj [128,512] (1) + transpose [128,256] (1).
"""

import os
import sys
import numpy as np

for _p in ("/opt/trn_rl_repo", "/opt/pypackages"):
    if _p not in sys.path:
        sys.path.insert(0, _p)

B, N, C = 2, 2048, 1024
H, D = 16, 64
HG = 4            # heads per core
NCH = 16          # token chunks of 128
QB = 4            # q blocks of 512
KTC = 16          # key chunks of 128
EPS = 1e-6

_PROG = None      # cached compiled Bass program
LAST_EXEC_NS = None
LAST_PROFILE = None


def _build_program():
    import concourse.bass as bass
    import concourse.bacc as bacc
    import concourse.tile as tile
    import concourse.mybir as mybir

    F32 = mybir.dt.float32
    BF = mybir.dt.bfloat16
    AF = mybir.ActivationFunctionType
    OP = mybir.AluOpType

    nc = bacc.Bacc("TRN2", target_bir_lowering=False, debug=False, num_devices=8)

    xT = nc.dram_tensor("xT", [8, 128, N], BF, kind="ExternalInput")
    ctxT = nc.dram_tensor("ctxT", [8, 128, N], BF, kind="ExternalInput")
    wq = nc.dram_tensor("wq", [8, 128, 256], BF, kind="ExternalInput")
    wg = nc.dram_tensor("wg", [8, 128, 256], BF, kind="ExternalInput")
    wkv = nc.dram_tensor("wkv", [8, 128, 512], BF, kind="ExternalInput")
    wo = nc.dram_tensor("wo", [2, 128, 1024], BF, kind="ExternalInput")
    cosq = nc.dram_tensor("cosq", [N, D], BF, kind="ExternalInput")
    ssinq = nc.dram_tensor("ssinq", [N, D], BF, kind="ExternalInput")
    cosk = nc.dram_tensor("cosk", [N, D], BF, kind="ExternalInput")
    ssink = nc.dram_tensor("ssink", [N, D], BF, kind="ExternalInput")
    part = nc.dram_tensor("part", [N, C], F32, kind="ExternalOutput")

    def bcast4(ap):
        # [128, 64] -> [128, 4, 64] with step-0 middle dim (read-broadcast)
        return bass.AP(tensor=ap.tensor, offset=ap.offset,
                       ap=[ap.ap[0], [0, 4], ap.ap[1]])

    def bcast_rstd(ap):
        # [128, 4] -> [128, 64, 4] d-major view broadcasting each head's
        # scalar over d (keeps the zero stride out of the innermost dim)
        return bass.AP(tensor=ap.tensor, offset=ap.offset,
                       ap=[ap.ap[0], [0, 64], ap.ap[1]])

    def dmajor(ap):
        # [128, 4, 64] -> [128, 64, 4] transposed free-dim view
        p, hdim, ddim = ap.ap
        return bass.AP(tensor=ap.tensor, offset=ap.offset,
                       ap=[p, ddim, hdim])

    def swap_view(ap):
        # ap: [128, 4, 64] contiguous -> per head read order d+32..d+63, d..d+31
        p, hdim, ddim = ap.ap
        return bass.AP(tensor=ap.tensor, offset=ap.offset + 32 * ddim[0],
                       ap=[p, hdim, [-32 * ddim[0], 2], [ddim[0], 32]])

    with tile.TileContext(nc) as tc:
        import contextlib
        with contextlib.ExitStack() as ctx:
            singles = ctx.enter_context(tc.tile_pool(name="singles", bufs=1))
            slices = ctx.enter_context(tc.tile_pool(name="slices", bufs=3))
            work = ctx.enter_context(tc.tile_pool(name="work", bufs=2))
            persist = ctx.enter_context(tc.tile_pool(name="persist", bufs=1))
            exps_p = ctx.enter_context(tc.tile_pool(name="exps", bufs=6))
            gat_p = ctx.enter_context(tc.tile_pool(name="gat", bufs=2))
            pools = {}  # phase-dependent PSUM pools: 'proj' and 'tp'

            # ---- first-needed weights/tables (DMA order matters) ----
            wkv_sb = singles.tile([128, 8, 512], BF)
            nc.sync.dma_start(out=wkv_sb, in_=wkv.ap().rearrange("c p f -> p c f"))
            ck_sb = singles.tile([128, NCH, D], BF)
            nc.sync.dma_start(out=ck_sb, in_=cosk.ap().rearrange("(i p) d -> p i d", p=128))
            sk_sb = singles.tile([128, NCH, D], BF)
            nc.sync.dma_start(out=sk_sb, in_=ssink.ap().rearrange("(i p) d -> p i d", p=128))
            from concourse.masks import make_identity
            ident = singles.tile([128, 128], BF)
            make_identity(nc, ident)
            ones1 = singles.tile([128, 1], BF)
            nc.vector.memset(ones1, 1.0)
            ones2 = singles.tile([128, 64], BF)
            nc.vector.memset(ones2, 1.0)
            eps_sb = singles.tile([128, 1], F32)
            nc.vector.memset(eps_sb, EPS)
            I32 = mybir.dt.int32
            magic_sb = singles.tile([128, 4], I32)
            nc.vector.memset(magic_sb, 0x5F3759DF)

            # ---- persistent intermediates ----
            pairK = persist.tile([128, 2, N], BF, tag="pairK")
            pairQb = [persist.tile([128, 2, 512], BF, tag=f"pairQ{q}",
                                   name=f"pairQ{q}") for q in range(QB)]
            v_sb = persist.tile([128, KTC, 4, 64], BF, tag="v_sb")
            graw = [persist.tile([128, 2, 512], BF, tag=f"graw{q}",
                                 name=f"graw{q}") for q in range(QB)]
            A_sb = [persist.tile([128, 2, 512], BF, tag=f"A{q}",
                                 name=f"A{q}") for q in range(QB)]

            def qk_stages(mode, sl, ns, j, w_sb, wcols, cos_t, sin_t, dst,
                          dslice):
                """Staged project+norm+rope+transpose for chunk j of q
                (mode='q') or k/v. Returns [s0..s4]: s0-s2 issue 2-3 proj MMs
                each, s3 the last MMs plus the ACT/DVE norm+rope chain, s4
                (schedule >=2 steps later) the PE transposes + evac so the PE
                queue never waits on the cross-engine chain.

                mode 'kv' (ACT idle) -> evacs on ACT; 'q' (attention-bound
                ACT) -> evacs on DVE.
                """
                box = {}

                def mms(c0, c1):
                    for c in range(c0, c1):
                        nc.tensor.matmul(box["ps"][:, :wcols],
                                         sl[:, c, ns * 128:(ns + 1) * 128],
                                         w_sb[:, c, :],
                                         start=(c == 0), stop=(c == 7))

                def s0():
                    box["ps"] = pools["proj"].tile([128, 512], F32,
                                                   tag="proj", name="ps")
                    mms(0, 2)

                def s3():
                    mms(6, 8)
                    ps = box["ps"]
                    if mode == "q":
                        # early evac: frees the shared PSUM bank for the next
                        # interleaved consumer ~0.5us after the last MM
                        qsb = work.tile([128, 256], F32, tag="qsb")
                        nc.vector.tensor_copy(out=qsb, in_=ps[:, 0:256])
                        qpart = qsb
                    else:
                        qpart = ps[:, 0:256]
                    qhd = qpart.rearrange("p (h d) -> p h d", h=4)
                    # variance (zero-mean folded into host-centered weights)
                    sqv = work.tile([128, 256], F32, tag="sq")
                    nc.scalar.activation(out=sqv, in_=qpart, func=AF.Square)
                    ssum = work.tile([128, 4], F32, tag="ssum")
                    nc.vector.tensor_reduce(
                        out=ssum, in_=sqv.rearrange("p (h d) -> p h d", h=4),
                        axis=mybir.AxisListType.X, op=OP.add)
                    if mode == "kv":
                        sdev = work.tile([128, 4], F32, tag="sdev")
                        nc.scalar.activation(out=sdev, in_=ssum, func=AF.Sqrt,
                                             scale=1.0 / 64.0, bias=eps_sb)
                        rstd = work.tile([128, 4], F32, tag="rstd")
                        nc.vector.reciprocal(out=rstd, in_=sdev)
                    else:
                        # rsqrt on DVE (Newton, bit-trick seed): the ACT Sqrt
                        # lives in a different function table than Exp and a
                        # mid-attention table reload costs ~1.5us + thrash
                        var = work.tile([128, 4], F32, tag="var")
                        nc.vector.tensor_scalar(out=var, in0=ssum,
                                                scalar1=1.0 / 64.0,
                                                scalar2=EPS,
                                                op0=OP.mult, op1=OP.add)
                        ti = work.tile([128, 4], I32, tag="ti")
                        nc.vector.tensor_scalar(
                            out=ti, in0=var.bitcast(I32), scalar1=1,
                            scalar2=None, op0=OP.logical_shift_right)
                        y0i = work.tile([128, 4], I32, tag="y0i")
                        nc.vector.tensor_tensor(out=y0i, in0=magic_sb,
                                                in1=ti, op=OP.subtract)
                        hv = work.tile([128, 4], F32, tag="hv")
                        nc.vector.tensor_scalar(out=hv, in0=var, scalar1=0.5,
                                                scalar2=None, op0=OP.mult)
                        y = y0i.bitcast(F32)
                        for it in range(2):
                            aa = work.tile([128, 4], F32, tag=f"aa{it}")
                            nc.vector.tensor_tensor(out=aa, in0=y, in1=y,
                                                    op=OP.mult)
                            bb = work.tile([128, 4], F32, tag=f"bb{it}")
                            nc.vector.tensor_tensor(out=bb, in0=aa, in1=hv,
                                                    op=OP.mult)
                            cc = work.tile([128, 4], F32, tag=f"cc{it}")
                            nc.vector.tensor_scalar(out=cc, in0=bb,
                                                    scalar1=-1.0, scalar2=1.5,
                                                    op0=OP.mult, op1=OP.add)
                            yn = work.tile([128, 4], F32, tag=f"yn{it}")
                            nc.vector.tensor_tensor(out=yn, in0=y, in1=cc,
                                                    op=OP.mult)
                            y = yn
                        rstd = y
                    qn = work.tile([128, 4, 64], BF, tag="qn")
                    nc.vector.tensor_tensor(out=dmajor(qn), in0=dmajor(qhd),
                                            in1=bcast_rstd(rstd), op=OP.mult)
                    # rope: qr = qn*cos + swap(qn)*ssin (sign folded in ssin)
                    t1 = work.tile([128, 4, 64], BF, tag="t1")
                    nc.vector.tensor_tensor(out=t1, in0=qn, in1=bcast4(cos_t),
                                            op=OP.mult)
                    t2 = work.tile([128, 4, 64], BF, tag="t2")
                    nc.vector.tensor_tensor(out=t2, in0=swap_view(qn),
                                            in1=bcast4(sin_t), op=OP.mult)
                    qr = work.tile([128, 4, 64], BF, tag="qr")
                    nc.vector.tensor_tensor(out=qr, in0=t1, in1=t2, op=OP.add)
                    box["qr"] = qr
                    if mode == "kv":
                        # v evac on ACT
                        nc.scalar.activation(
                            out=v_sb[:, j, :, :],
                            in_=ps[:, 256:512].rearrange("p (h d) -> p h d",
                                                         h=4),
                            func=AF.Copy)

                def s4():
                    # PE transposes: both head-pairs into ONE psum bank
                    # (T1 start clears the bank; T2 must not re-clear)
                    qr = box["qr"]
                    pst = pools["mk_tp"]()
                    for p in range(2):
                        nc.tensor.matmul(
                            pst[:, p * 128:(p + 1) * 128],
                            qr[:, 2 * p:2 * p + 2, :]
                            .rearrange("p a b -> p (a b)"),
                            ident, is_transpose=True,
                            start=(p == 0), stop=(p == 1),
                            skip_group_check=True)
                    dst_ap = dst[:, :, dslice]
                    src_ap = pst.rearrange("p (a b) -> p a b", a=2)
                    if mode == "kv":
                        nc.scalar.activation(out=dst_ap, in_=src_ap,
                                             func=AF.Copy)
                    else:
                        nc.vector.tensor_copy(out=dst_ap, in_=src_ap)

                return [s0, lambda: mms(2, 4), lambda: mms(4, 6), s3, s4]

            def kv_stages(c_sl, ns, j):
                return qk_stages("kv", c_sl, ns, j, wkv_sb, 512,
                                 ck_sb[:, j, :], sk_sb[:, j, :],
                                 pairK, slice(j * 128, (j + 1) * 128))

            def q_stages(x_sl, ns, qcn):
                j = qcn * 4 + ns
                return qk_stages("q", x_sl, ns, j, wq_sb, 256,
                                 cq_sb[:, j, :], sq_sb[:, j, :],
                                 pairQb[qcn], slice(ns * 128, (ns + 1) * 128))

            def gate_stages(x_sl, gfc, qcn):
                """Gate projection split into 4 hook steps of 2 MMs."""
                box = {}

                def mms(c0, c1):
                    if "psg" not in box:
                        box["psg"] = pools["proj"].tile([128, 512], F32,
                                                        tag="proj",
                                                        name="psg")
                    for c in range(c0, c1):
                        nc.tensor.matmul(
                            box["psg"], wg_sb[:, c, gfc * 128:(gfc + 1) * 128],
                            x_sl[:, c, :], start=(c == 0), stop=(c == 7))

                def last():
                    mms(6, 8)
                    nc.vector.tensor_copy(out=graw[qcn][:, gfc, :],
                                          in_=box["psg"])

                return [lambda: mms(0, 2), lambda: mms(2, 4),
                        lambda: mms(4, 6), last]

            # ================= phase A: KV + Q block 0 =================
            wq_sb = singles.tile([128, 8, 256], BF)
            wg_sb = singles.tile([128, 8, 256], BF)
            wo_sb = singles.tile([128, 2, 1024], BF)
            cq_sb = singles.tile([128, NCH, D], BF)
            sq_sb = singles.tile([128, NCH, D], BF)

            def load_x_slice(qcn):
                x_sl = slices.tile([128, 8, 512], BF, tag="slice", name="x_sl")
                nc.sync.dma_start(
                    out=x_sl,
                    in_=xT.ap()[:, :, qcn * 512:(qcn + 1) * 512]
                    .rearrange("c p n -> p c n"))
                return x_sl

            with tc.tile_pool(name="psP", bufs=4, space="PSUM") as psP, \
                 tc.tile_pool(name="psTT", bufs=2, space="PSUM") as psTT:
                pools["proj"] = psP
                pools["mk_tp"] = lambda: psTT.tile([128, 256], BF, tag="tp",
                                                   name="pst")
                prev_s4 = None
                for qc4 in range(4):
                    c_sl = slices.tile([128, 8, 512], BF, tag="slice", name="c_sl")
                    nc.sync.dma_start(
                        out=c_sl,
                        in_=ctxT.ap()[:, :, qc4 * 512:(qc4 + 1) * 512]
                        .rearrange("c p n -> p c n"))
                    if qc4 == 0:
                        # q-side weights/tables behind the first ctx slice
                        nc.sync.dma_start(out=wq_sb, in_=wq.ap().rearrange("c p f -> p c f"))
                        nc.sync.dma_start(out=wg_sb, in_=wg.ap().rearrange("c p f -> p c f"))
                        nc.sync.dma_start(out=cq_sb, in_=cosq.ap().rearrange("(i p) d -> p i d", p=128))
                        nc.sync.dma_start(out=sq_sb, in_=ssinq.ap().rearrange("(i p) d -> p i d", p=128))
                        nc.sync.dma_start(out=wo_sb, in_=wo.ap().rearrange("c p f -> p c f"))
                    for ns in range(4):
                        st = kv_stages(c_sl, ns, qc4 * 4 + ns)
                        for s in st[:4]:
                            s()
                        # previous chunk's transposes after this chunk's MMs:
                        # by then its norm/rope chain has drained, so the PE
                        # never head-of-line blocks on DVE
                        if prev_s4 is not None:
                            prev_s4()
                        prev_s4 = st[4]

                x_sl0 = load_x_slice(0)
                for ns in range(4):
                    st = q_stages(x_sl0, ns, 0)
                    for s in st[:4]:
                        s()
                    if prev_s4 is not None:
                        prev_s4()
                    prev_s4 = st[4]
                for gfc in range(2):
                    for s in gate_stages(x_sl0, gfc, 0):
                        s()
                prev_s4()

            # ================= attention + interleaved next-block proj =======
            # Fully software-pipelined flat loop over (qc, pair, k-chunk):
            # per step: flush oldest pending attn@v/denominator MMs, then
            # score MMs + exp, then hooks (gating / out-proj / next-block
            # projections) so exp never waits behind bunched boundary work.
            psSC = ctx.enter_context(tc.tile_pool(name="psSC", bufs=2, space="PSUM"))
            psACC = ctx.enter_context(tc.tile_pool(name="psACC", bufs=3, space="PSUM"))
            psA = ctx.enter_context(tc.tile_pool(name="psA", bufs=1, space="PSUM"))
            pools["proj"] = psA
            # interleaved-phase transposes reuse the current step's
            # just-consumed scores bank (no extra alloc: ring parity and the
            # exp pipeline are preserved; WAR on the exp read is tracked)
            cur_sc = {}
            pools["mk_tp"] = lambda: cur_sc["ps"].bitcast(BF)[:, 0:256]

            state = {}   # (qc, p) -> (ao, dn) PSUM tiles

            def flush_attn(eS, qc, p, k):
                if (qc, p) not in state:
                    # dn before ao: ring slot rotation then ties each new
                    # alloc to the oldest pair's earliest-finishing readers
                    dn_p = psACC.tile([128, 512], F32, tag="acc", name="dn")
                    ao_p = psACC.tile([128, 512], F32, tag="acc", name="ao")
                    state[(qc, p)] = (ao_p, dn_p)
                ao_p, dn_p = state[(qc, p)]
                # first MM of the k==0 group clears the whole bank;
                # the second must NOT re-clear (would drop the
                # first's has_written bits) -> start only on MM1.
                st = (k == 0)
                sp = (k == KTC - 1)
                nc.tensor.matmul(ao_p[0:64, :], v_sb[:, k, 2 * p, :],
                                 eS[:, 0, :], start=st, stop=sp,
                                 tile_position=(0, 0),
                                 skip_group_check=True)
                nc.tensor.matmul(ao_p[64:128, :], v_sb[:, k, 2 * p + 1, :],
                                 eS[:, 1, :], start=st, stop=sp,
                                 tile_position=(0, 64),
                                 skip_group_check=True)
                # denominator rows 0 and 64: opposite column quadrants so the
                # two M=1 streams run concurrently
                nc.tensor.matmul(dn_p[0:1, :], ones1, eS[:, 0, :],
                                 start=st, stop=sp,
                                 tile_position=(0, 0),
                                 skip_group_check=True)
                nc.tensor.matmul(dn_p[64:65, :], ones1, eS[:, 1, :],
                                 start=st, stop=sp,
                                 tile_position=(0, 64),
                                 skip_group_check=True)

            def gating(qc, p):
                # (DVE; reciprocal-denominator broadcast via PE ones matmul)
                ao_p, dn_p = state.pop((qc, p))
                gs = gat_p.tile([128, 512], F32, tag="gs")
                nc.scalar.activation(out=gs, in_=graw[qc][:, p, :],
                                     func=AF.Tanh, scale=0.5)
                sig = gat_p.tile([128, 512], F32, tag="sig")
                nc.vector.tensor_scalar(out=sig, in0=gs, scalar1=0.5,
                                        scalar2=0.5, op0=OP.mult,
                                        op1=OP.add)
                dns = gat_p.tile([128, 512], BF, tag="dns")
                nc.vector.tensor_copy(out=dns[0:1, :], in_=dn_p[0:1, :])
                nc.vector.tensor_copy(out=dns[64:65, :], in_=dn_p[64:65, :])
                rbc = pools["proj"].tile([128, 512], F32, tag="proj",
                                         name="rbc")
                nc.tensor.matmul(rbc[0:64, :], ones2[0:1, :], dns[0:1, :],
                                 start=True, stop=True,
                                 tile_position=(0, 0))
                nc.tensor.matmul(rbc[64:128, :], ones2[64:65, :],
                                 dns[64:65, :], start=True, stop=True,
                                 tile_position=(64, 64))
                rec = gat_p.tile([128, 512], F32, tag="rec")
                nc.vector.reciprocal(out=rec, in_=rbc)
                m = gat_p.tile([128, 512], F32, tag="m")
                nc.vector.tensor_tensor(out=m, in0=sig, in1=rec, op=OP.mult)
                nc.vector.tensor_tensor(out=A_sb[qc][:, p, :], in0=ao_p,
                                        in1=m, op=OP.mult)

            def outproj_piece(qc, i):
                # accumulates in the just-consumed scores tile's second bank:
                # no pool alloc, so the scores ring and proj bank are untouched
                nk, oc = divmod(i, 2)
                n1 = qc * 4 + nk
                po = cur_sc["ps"][:, 512:1024]
                for fc in range(2):
                    nc.tensor.matmul(
                        po,
                        A_sb[qc][:, fc, nk * 128:(nk + 1) * 128],
                        wo_sb[:, fc, oc * 512:(oc + 1) * 512],
                        start=(fc == 0), stop=(fc == 1),
                        skip_group_check=True)
                ev = gat_p.tile([128, 512], F32, tag="ev")
                nc.vector.tensor_copy(out=ev, in_=po)
                nc.sync.dma_start(
                    out=part.ap()[n1 * 128:(n1 + 1) * 128,
                                  oc * 512:(oc + 1) * 512],
                    in_=ev)

            hooks = {}

            def add_hook(qc, p, k, fn):
                hooks.setdefault((qc, p, k), []).append(fn)

            def at(qc, p, k):
                # normalize a possibly-overflowing (qc, p, k) step address
                t = qc * 32 + p * 16 + k
                return t // 32, (t % 32) // 16, t % 16

            def add_stages(first, stages, gap=1):
                """Register stage list at consecutive steps (gap apart); the
                final stage (transpose+evac) runs 3 steps after the prior."""
                qc, p, k = first
                for i, s in enumerate(stages[:-1]):
                    add_hook(*at(qc, p, k + i * gap), s)
                add_hook(*at(qc, p, k + (len(stages) - 2) * gap + 3),
                         stages[-1])

            for qc in range(4):
                # gating as soon as the pair's accumulation completes
                # (flushes trail the score loop by 3 steps)
                add_hook(qc, 1, 2, lambda qc=qc: gating(qc, 0))
                if qc > 0:
                    add_hook(qc, 0, 2, lambda qc=qc: gating(qc - 1, 1))
                    for i in range(8):
                        add_hook(qc, 0, 3 + i,
                                 lambda qc=qc, i=i: outproj_piece(qc - 1, i))
                if qc + 1 < QB:
                    # next q-block projections spread through this window
                    qcn = qc + 1
                    b = []
                    add_hook(qc, 0, 1, lambda qcn=qcn, b=b:
                             b.append(load_x_slice(qcn)))

                    def lazy_stages(factory, n):
                        box2 = {}

                        def run(i):
                            if "s" not in box2:
                                box2["s"] = factory()
                            box2["s"][i]()

                        return [lambda i=i: run(i) for i in range(n)]

                    def mk(ns, qcn=qcn, b=b):
                        return lazy_stages(
                            lambda: q_stages(b[0], ns, qcn), 5)

                    add_stages((qc, 0, 11), mk(0))
                    add_stages((qc, 0, 15), mk(1))
                    add_stages((qc, 1, 3), mk(2))
                    add_stages((qc, 1, 7), mk(3))

                    def mkg(gfc, qcn=qcn, b=b):
                        return lazy_stages(
                            lambda: gate_stages(b[0], gfc, qcn), 4)

                    g0 = mkg(0)
                    g1 = mkg(1)
                    for i in range(4):
                        add_hook(*at(qc, 1, 11 + i), g0[i])
                        add_hook(*at(qc, 1, 15 + i), g1[i])

            pend = []  # (expS tile, qc, p, ktc) awaiting attn MMs
            for qc in range(4):
                for p in range(2):
                    for k in range(KTC):
                        if len(pend) >= 3:
                            flush_attn(*pend.pop(0))
                        ksl = slice(k * 128, (k + 1) * 128)
                        ps = psSC.tile([128, 1024], F32, tag="sc", name="sc")
                        nc.tensor.matmul(ps[:, 0:512],
                                         pairK[0:64, p, ksl],
                                         pairQb[qc][0:64, p, :],
                                         start=True, stop=True,
                                         tile_position=(0, 0))
                        nc.tensor.matmul(ps[:, 512:1024],
                                         pairK[64:128, p, ksl],
                                         pairQb[qc][64:128, p, :],
                                         start=True, stop=True,
                                         tile_position=(64, 0))
                        eS = exps_p.tile([128, 2, 512], BF, tag="expS",
                                         name="eS")
                        nc.scalar.activation(
                            out=eS.rearrange("p a b -> p (a b)"), in_=ps,
                            func=AF.Exp, scale=0.125)
                        pend.append((eS, qc, p, k))
                        cur_sc["ps"] = ps
                        for fn in hooks.pop((qc, p, k), ()):
                            fn()
            for e in pend:
                flush_attn(*e)
            gating(3, 1)
            for i in range(8):
                outproj_piece(3, i)

    nc.compile()
    return nc


def _prep_core(inputs, b, g, bf16):
    x = np.asarray(inputs["x"][b], dtype=np.float32)
    ctx = np.asarray(inputs["context"][b], dtype=np.float32)
    Wq = np.asarray(inputs["Wq"], dtype=np.float32).reshape(H, 2 * D, C)
    Wkv = np.asarray(inputs["Wkv"], dtype=np.float32).reshape(H, 2 * D, C)
    Wo = np.asarray(inputs["Wo"], dtype=np.float32)
    cos = np.asarray(inputs["cos"][b], dtype=np.float32)
    sin = np.asarray(inputs["sin"][b], dtype=np.float32)
    qw = np.asarray(inputs["q_norm_w"], dtype=np.float32)
    kw = np.asarray(inputs["k_norm_w"], dtype=np.float32)

    hs = slice(HG * g, HG * g + HG)
    qr = Wq[hs, :D, :]                       # [4, D, C]
    qr = qr - qr.mean(axis=1, keepdims=True)
    gr = Wq[hs, D:, :]
    kr = Wkv[hs, :D, :]
    kr = kr - kr.mean(axis=1, keepdims=True)
    vr = Wkv[hs, D:, :]

    sgn = np.where(np.arange(D) < D // 2, -1.0, 1.0).astype(np.float32)
    wswap = lambda w: np.concatenate([w[D // 2:], w[:D // 2]])

    return {
        "xT": np.ascontiguousarray(x.T).reshape(8, 128, N).astype(bf16),
        "ctxT": np.ascontiguousarray(ctx.T).reshape(8, 128, N).astype(bf16),
        "wq": np.ascontiguousarray(
            qr.reshape(HG * D, C).T).reshape(8, 128, 256).astype(bf16),
        "wg": np.ascontiguousarray(
            gr.reshape(HG * D, C).T).reshape(8, 128, 256).astype(bf16),
        "wkv": np.ascontiguousarray(
            np.concatenate([kr.reshape(HG * D, C), vr.reshape(HG * D, C)], 0).T
        ).reshape(8, 128, 512).astype(bf16),
        "wo": np.ascontiguousarray(
            Wo[:, 256 * g:256 * (g + 1)].T).reshape(2, 128, C).astype(bf16),
        "cosq": (cos * qw[None, :]).astype(bf16),
        "ssinq": (sin * sgn[None, :] * wswap(qw)[None, :]).astype(bf16),
        "cosk": (cos * kw[None, :]).astype(bf16),
        "ssink": (sin * sgn[None, :] * wswap(kw)[None, :]).astype(bf16),
    }


def kernel(**inputs):
    global _PROG, LAST_EXEC_NS, LAST_PROFILE
    import ml_dtypes
    bf16 = ml_dtypes.bfloat16

    if _PROG is None:
        _PROG = _build_program()
    nc = _PROG

    in_maps = [_prep_core(inputs, core // 4, core % 4, bf16) for core in range(8)]

    trace = bool(os.environ.get("BASS_KERNEL_TRACE"))
    kw = {}
    if trace:
        import types
        from trn_agent_boot.trn_boot import _ntff_profile_via_ctypes
        hook = _ntff_profile_via_ctypes('/opt/axon/libaxon_pjrt.so')
        mod = types.ModuleType('antenv.axon_hooks')
        mod.get_axon_ntff_profile_hook = lambda: hook
        sys.modules['antenv.axon_hooks'] = mod
        from concourse import bass_utils
        bass_utils.upload_artifacts = lambda tmpdir: tmpdir
        kw = dict(trace=True, tmpdir=os.environ.get("BASS_KERNEL_TRACE_DIR"))

    from concourse.bass_utils import run_bass_kernel_spmd
    res = run_bass_kernel_spmd(nc, in_maps, core_ids=list(range(8)), **kw)
    LAST_EXEC_NS = res.exec_time_ns
    LAST_PROFILE = res.profile_json

    bo = np.asarray(inputs["bo"], dtype=np.float32)
    out = np.zeros((B, N, C), dtype=np.float32)
    for core in range(8):
        out[core // 4] += res.results[core]["part"]
    out += bo[None, None, :]
    return out


# revision 36
# speedup vs baseline: 1.2790x; 1.0179x over previous
"""Trainium2 Bass kernel for nn_CrossAttention (B=2, N=2048, C=1024, H=16, D=64).

Sharding: 8 cores = 2 batches x 4 head-groups (4 heads each).
Each core computes its head-group's attention + a partial output projection;
the host sums the 4 partials per batch and adds the bias.

Device pipeline per core (v2 - engine-balanced, phase-interleaved):
  All matmul operands bf16 (f32r measured ~1.7x slower per row on HW).
  KV phase: project+norm+rope all 16 context chunks. Variance via ACT Square
    (ACT idle here), PSUM evacs on ACT, DVE does reduce/rstd/rope (rstd applied
    via one broadcast-view tensor_tensor).
  Q chunks + gate for q-block 0, then per q-block qc: attention pair loops
    (paired score matmuls -> ACT exp from 2-bank PSUM, scale=1/8, no max
    subtraction - rms-normed q,k bound |score| <= 8 -> paired attn@v + M=1
    ones matmuls for denominators), with the Q/gate projections for qc+1
    INTERLEAVED into the score loops so the exp-bound attention phase and the
    DVE/PE-bound projection phase overlap. Q-phase evacs/square go to DVE
    (ACT is the bottleneck during attention). Gating + out-proj evacs on DVE.
  PSUM budget (8 banks): scores 2x[128,1024] (4) + acc ao/dn/outproj
    rotation 2x[128,512] (2) + proj [128,512] (1) + transpose [128,256] (1).
"""

import os
import sys
import numpy as np

for _p in ("/opt/trn_rl_repo", "/opt/pypackages"):
    if _p not in sys.path:
        sys.path.insert(0, _p)

B, N, C = 2, 2048, 1024
H, D = 16, 64
HG = 4            # heads per core
NCH = 16          # token chunks of 128
QB = 4            # q blocks of 512
KTC = 16          # key chunks of 128
EPS = 1e-6

_PROG = None      # cached compiled Bass program
LAST_EXEC_NS = None
LAST_PROFILE = None


def _build_program():
    import concourse.bass as bass
    import concourse.bacc as bacc
    import concourse.tile as tile
    import concourse.mybir as mybir

    F32 = mybir.dt.float32
    BF = mybir.dt.bfloat16
    AF = mybir.ActivationFunctionType
    OP = mybir.AluOpType

    nc = bacc.Bacc("TRN2", target_bir_lowering=False, debug=False, num_devices=8)

    xT = nc.dram_tensor("xT", [8, 128, N], BF, kind="ExternalInput")
    ctxT = nc.dram_tensor("ctxT", [8, 128, N], BF, kind="ExternalInput")
    wq = nc.dram_tensor("wq", [128, 8, 256], BF, kind="ExternalInput")
    wg = nc.dram_tensor("wg", [128, 8, 256], BF, kind="ExternalInput")
    wkv = nc.dram_tensor("wkv", [128, 8, 512], BF, kind="ExternalInput")
    wo = nc.dram_tensor("wo", [128, 2, 1024], BF, kind="ExternalInput")
    cosq = nc.dram_tensor("cosq", [128, NCH, D], BF, kind="ExternalInput")
    ssinq = nc.dram_tensor("ssinq", [128, NCH, D], BF, kind="ExternalInput")
    cosk = nc.dram_tensor("cosk", [128, NCH, D], BF, kind="ExternalInput")
    ssink = nc.dram_tensor("ssink", [128, NCH, D], BF, kind="ExternalInput")
    part = nc.dram_tensor("part", [N, C], F32, kind="ExternalOutput")

    def bcast4(ap):
        # [128, 64] -> [128, 4, 64] with step-0 middle dim (read-broadcast)
        return bass.AP(tensor=ap.tensor, offset=ap.offset,
                       ap=[ap.ap[0], [0, 4], ap.ap[1]])

    def bcast_rstd(ap):
        # [128, 4] -> [128, 64, 4] d-major view broadcasting each head's
        # scalar over d (keeps the zero stride out of the innermost dim)
        return bass.AP(tensor=ap.tensor, offset=ap.offset,
                       ap=[ap.ap[0], [0, 64], ap.ap[1]])

    def dmajor(ap):
        # [128, 4, 64] -> [128, 64, 4] transposed free-dim view
        p, hdim, ddim = ap.ap
        return bass.AP(tensor=ap.tensor, offset=ap.offset,
                       ap=[p, ddim, hdim])

    def swap_view(ap):
        # ap: [128, 4, 64] contiguous -> per head read order d+32..d+63, d..d+31
        p, hdim, ddim = ap.ap
        return bass.AP(tensor=ap.tensor, offset=ap.offset + 32 * ddim[0],
                       ap=[p, hdim, [-32 * ddim[0], 2], [ddim[0], 32]])

    with tile.TileContext(nc) as tc:
        import contextlib
        with contextlib.ExitStack() as ctx:
            singles = ctx.enter_context(tc.tile_pool(name="singles", bufs=1))
            slices = ctx.enter_context(tc.tile_pool(name="slices", bufs=3))
            work = ctx.enter_context(tc.tile_pool(name="work", bufs=2))
            persist = ctx.enter_context(tc.tile_pool(name="persist", bufs=1))
            exps_p = ctx.enter_context(tc.tile_pool(name="exps", bufs=6))
            gat_p = ctx.enter_context(tc.tile_pool(name="gat", bufs=2))
            pools = {}  # phase-dependent PSUM pools: 'proj' and 'tp'

            # ---- first-needed weights/tables (DMA order matters) ----
            wkv_sb = singles.tile([128, 8, 512], BF)
            nc.sync.dma_start(out=wkv_sb, in_=wkv.ap())
            ck_sb = singles.tile([128, NCH, D], BF)
            nc.sync.dma_start(out=ck_sb, in_=cosk.ap())
            sk_sb = singles.tile([128, NCH, D], BF)
            nc.sync.dma_start(out=sk_sb, in_=ssink.ap())
            from concourse.masks import make_identity
            ident = singles.tile([128, 128], BF)
            make_identity(nc, ident)
            ones1 = singles.tile([128, 1], BF)
            nc.vector.memset(ones1, 1.0)
            ones2 = singles.tile([128, 64], BF)
            nc.vector.memset(ones2, 1.0)
            eps_sb = singles.tile([128, 1], F32)
            nc.vector.memset(eps_sb, EPS)
            I32 = mybir.dt.int32
            magic_sb = singles.tile([128, 4], I32)
            nc.vector.memset(magic_sb, 0x5F3759DF)

            # ---- persistent intermediates ----
            pairK = persist.tile([128, 2, N], BF, tag="pairK")
            pairQb = [persist.tile([128, 2, 512], BF, tag=f"pairQ{q}",
                                   name=f"pairQ{q}") for q in range(QB)]
            v_sb = persist.tile([128, KTC, 4, 64], BF, tag="v_sb")
            graw = [persist.tile([128, 2, 512], BF, tag=f"graw{q}",
                                 name=f"graw{q}") for q in range(QB)]
            A_sb = [persist.tile([128, 2, 512], BF, tag=f"A{q}",
                                 name=f"A{q}") for q in range(QB)]

            def qk_stages(mode, sl, ns, j, w_sb, wcols, cos_t, sin_t, dst,
                          dslice):
                """Staged project+norm+rope+transpose for chunk j of q
                (mode='q') or k/v. Returns [s0..s4]: s0-s2 issue 2-3 proj MMs
                each, s3 the last MMs plus the ACT/DVE norm+rope chain, s4
                (schedule >=2 steps later) the PE transposes + evac so the PE
                queue never waits on the cross-engine chain.

                mode 'kv' (ACT idle) -> evacs on ACT; 'q' (attention-bound
                ACT) -> evacs on DVE.
                """
                box = {}

                def mms(c0, c1):
                    for c in range(c0, c1):
                        nc.tensor.matmul(box["ps"][:, :wcols],
                                         sl[:, c, ns * 128:(ns + 1) * 128],
                                         w_sb[:, c, :],
                                         start=(c == 0), stop=(c == 7))

                def s0():
                    box["ps"] = pools["proj"].tile([128, 512], F32,
                                                   tag="proj", name="ps")
                    mms(0, 2)

                def s3():
                    mms(6, 8)
                    ps = box["ps"]
                    if mode == "q":
                        # early evac: frees the shared PSUM bank for the next
                        # interleaved consumer ~0.5us after the last MM
                        qsb = work.tile([128, 256], F32, tag="qsb")
                        nc.vector.tensor_copy(out=qsb, in_=ps[:, 0:256])
                        qpart = qsb
                    else:
                        qpart = ps[:, 0:256]
                    qhd = qpart.rearrange("p (h d) -> p h d", h=4)
                    # variance (zero-mean folded into host-centered weights)
                    sqv = work.tile([128, 256], F32, tag="sq")
                    nc.scalar.activation(out=sqv, in_=qpart, func=AF.Square)
                    ssum = work.tile([128, 4], F32, tag="ssum")
                    nc.vector.tensor_reduce(
                        out=ssum, in_=sqv.rearrange("p (h d) -> p h d", h=4),
                        axis=mybir.AxisListType.X, op=OP.add)
                    if mode == "kv":
                        sdev = work.tile([128, 4], F32, tag="sdev")
                        nc.scalar.activation(out=sdev, in_=ssum, func=AF.Sqrt,
                                             scale=1.0 / 64.0, bias=eps_sb)
                        rstd = work.tile([128, 4], F32, tag="rstd")
                        nc.vector.reciprocal(out=rstd, in_=sdev)
                    else:
                        # rsqrt on DVE (Newton, bit-trick seed): the ACT Sqrt
                        # lives in a different function table than Exp and a
                        # mid-attention table reload costs ~1.5us + thrash
                        var = work.tile([128, 4], F32, tag="var")
                        nc.vector.tensor_scalar(out=var, in0=ssum,
                                                scalar1=1.0 / 64.0,
                                                scalar2=EPS,
                                                op0=OP.mult, op1=OP.add)
                        ti = work.tile([128, 4], I32, tag="ti")
                        nc.vector.tensor_scalar(
                            out=ti, in0=var.bitcast(I32), scalar1=1,
                            scalar2=None, op0=OP.logical_shift_right)
                        y0i = work.tile([128, 4], I32, tag="y0i")
                        nc.vector.tensor_tensor(out=y0i, in0=magic_sb,
                                                in1=ti, op=OP.subtract)
                        hv = work.tile([128, 4], F32, tag="hv")
                        nc.vector.tensor_scalar(out=hv, in0=var, scalar1=0.5,
                                                scalar2=None, op0=OP.mult)
                        y = y0i.bitcast(F32)
                        for it in range(2):
                            aa = work.tile([128, 4], F32, tag=f"aa{it}")
                            nc.vector.tensor_tensor(out=aa, in0=y, in1=y,
                                                    op=OP.mult)
                            bb = work.tile([128, 4], F32, tag=f"bb{it}")
                            nc.vector.tensor_tensor(out=bb, in0=aa, in1=hv,
                                                    op=OP.mult)
                            cc = work.tile([128, 4], F32, tag=f"cc{it}")
                            nc.vector.tensor_scalar(out=cc, in0=bb,
                                                    scalar1=-1.0, scalar2=1.5,
                                                    op0=OP.mult, op1=OP.add)
                            yn = work.tile([128, 4], F32, tag=f"yn{it}")
                            nc.vector.tensor_tensor(out=yn, in0=y, in1=cc,
                                                    op=OP.mult)
                            y = yn
                        rstd = y
                    qn = work.tile([128, 4, 64], BF, tag="qn")
                    nc.vector.tensor_tensor(out=dmajor(qn), in0=dmajor(qhd),
                                            in1=bcast_rstd(rstd), op=OP.mult)
                    # rope: qr = qn*cos + swap(qn)*ssin (sign folded in ssin)
                    t1 = work.tile([128, 4, 64], BF, tag="t1")
                    nc.vector.tensor_tensor(out=t1, in0=qn, in1=bcast4(cos_t),
                                            op=OP.mult)
                    t2 = work.tile([128, 4, 64], BF, tag="t2")
                    nc.vector.tensor_tensor(out=t2, in0=swap_view(qn),
                                            in1=bcast4(sin_t), op=OP.mult)
                    qr = work.tile([128, 4, 64], BF, tag="qr")
                    nc.vector.tensor_tensor(out=qr, in0=t1, in1=t2, op=OP.add)
                    box["qr"] = qr
                    if mode == "kv":
                        # v evac on ACT
                        nc.scalar.activation(
                            out=v_sb[:, j, :, :],
                            in_=ps[:, 256:512].rearrange("p (h d) -> p h d",
                                                         h=4),
                            func=AF.Copy)

                def s4():
                    # PE transposes: both head-pairs into ONE psum bank
                    # (T1 start clears the bank; T2 must not re-clear)
                    qr = box["qr"]
                    pst = pools["mk_tp"]()
                    for p in range(2):
                        nc.tensor.matmul(
                            pst[:, p * 128:(p + 1) * 128],
                            qr[:, 2 * p:2 * p + 2, :]
                            .rearrange("p a b -> p (a b)"),
                            ident, is_transpose=True,
                            start=(p == 0), stop=(p == 1),
                            skip_group_check=True)
                    dst_ap = dst[:, :, dslice]
                    src_ap = pst.rearrange("p (a b) -> p a b", a=2)
                    if mode == "kv":
                        nc.scalar.activation(out=dst_ap, in_=src_ap,
                                             func=AF.Copy)
                    else:
                        nc.vector.tensor_copy(out=dst_ap, in_=src_ap)

                return [s0, lambda: mms(2, 4), lambda: mms(4, 6), s3, s4]

            def kv_stages(c_sl, ns, j):
                return qk_stages("kv", c_sl, ns, j, wkv_sb, 512,
                                 ck_sb[:, j, :], sk_sb[:, j, :],
                                 pairK, slice(j * 128, (j + 1) * 128))

            def q_stages(x_sl, ns, qcn):
                j = qcn * 4 + ns
                return qk_stages("q", x_sl, ns, j, wq_sb, 256,
                                 cq_sb[:, j, :], sq_sb[:, j, :],
                                 pairQb[qcn], slice(ns * 128, (ns + 1) * 128))

            def gate_stages(x_sl, gfc, qcn):
                """Gate projection split into 4 hook steps of 2 MMs."""
                box = {}

                def mms(c0, c1):
                    if "psg" not in box:
                        box["psg"] = pools["proj"].tile([128, 512], F32,
                                                        tag="proj",
                                                        name="psg")
                    for c in range(c0, c1):
                        nc.tensor.matmul(
                            box["psg"], wg_sb[:, c, gfc * 128:(gfc + 1) * 128],
                            x_sl[:, c, :], start=(c == 0), stop=(c == 7))

                def last():
                    mms(6, 8)
                    nc.vector.tensor_copy(out=graw[qcn][:, gfc, :],
                                          in_=box["psg"])

                return [lambda: mms(0, 2), lambda: mms(2, 4),
                        lambda: mms(4, 6), last]

            # ================= phase A: KV + Q block 0 =================
            wq_sb = singles.tile([128, 8, 256], BF)
            wg_sb = singles.tile([128, 8, 256], BF)
            wo_sb = singles.tile([128, 2, 1024], BF)
            cq_sb = singles.tile([128, NCH, D], BF)
            sq_sb = singles.tile([128, NCH, D], BF)

            def load_x_slice(qcn):
                x_sl = slices.tile([128, 8, 512], BF, tag="slice", name="x_sl")
                nc.sync.dma_start(
                    out=x_sl,
                    in_=xT.ap()[:, :, qcn * 512:(qcn + 1) * 512]
                    .rearrange("c p n -> p c n"))
                return x_sl

            with tc.tile_pool(name="psP", bufs=4, space="PSUM") as psP, \
                 tc.tile_pool(name="psTT", bufs=2, space="PSUM") as psTT:
                pools["proj"] = psP
                pools["mk_tp"] = lambda: psTT.tile([128, 256], BF, tag="tp",
                                                   name="pst")
                s4q = []
                for qc4 in range(4):
                    c_sl = slices.tile([128, 8, 512], BF, tag="slice", name="c_sl")
                    nc.sync.dma_start(
                        out=c_sl,
                        in_=ctxT.ap()[:, :, qc4 * 512:(qc4 + 1) * 512]
                        .rearrange("c p n -> p c n"))
                    if qc4 == 0:
                        # q-side weights/tables behind the first ctx slice
                        nc.sync.dma_start(out=wq_sb, in_=wq.ap())
                        nc.sync.dma_start(out=wg_sb, in_=wg.ap())
                        nc.sync.dma_start(out=cq_sb, in_=cosq.ap())
                        nc.sync.dma_start(out=sq_sb, in_=ssinq.ap())
                        nc.sync.dma_start(out=wo_sb, in_=wo.ap())
                    for ns in range(4):
                        st = kv_stages(c_sl, ns, qc4 * 4 + ns)
                        for s in st[:4]:
                            s()
                        # transposes lag 2 chunks so their norm/rope chains
                        # have fully drained (no PE head-of-line wait)
                        s4q.append(st[4])
                        if len(s4q) > 2:
                            s4q.pop(0)()

                x_sl0 = load_x_slice(0)
                for ns in range(4):
                    st = q_stages(x_sl0, ns, 0)
                    for s in st[:4]:
                        s()
                    s4q.append(st[4])
                    if len(s4q) > 2:
                        s4q.pop(0)()
                for gfc in range(2):
                    for s in gate_stages(x_sl0, gfc, 0):
                        s()
                for s in s4q:
                    s()

            # ================= attention + interleaved next-block proj =======
            # Fully software-pipelined flat loop over (qc, pair, k-chunk):
            # per step: flush oldest pending attn@v/denominator MMs, then
            # score MMs + exp, then hooks (gating / out-proj / next-block
            # projections) so exp never waits behind bunched boundary work.
            psSC = ctx.enter_context(tc.tile_pool(name="psSC", bufs=2, space="PSUM"))
            psACC = ctx.enter_context(tc.tile_pool(name="psACC", bufs=3, space="PSUM"))
            psA = ctx.enter_context(tc.tile_pool(name="psA", bufs=1, space="PSUM"))
            pools["proj"] = psA
            # interleaved-phase transposes reuse the current step's
            # just-consumed scores bank (no extra alloc: ring parity and the
            # exp pipeline are preserved; WAR on the exp read is tracked)
            cur_sc = {}
            pools["mk_tp"] = lambda: cur_sc["ps"].bitcast(BF)[:, 0:256]

            state = {}   # (qc, p) -> (ao, dn) PSUM tiles

            def flush_attn(eS, qc, p, k):
                if (qc, p) not in state:
                    # dn before ao: ring slot rotation then ties each new
                    # alloc to the oldest pair's earliest-finishing readers
                    dn_p = psACC.tile([128, 512], F32, tag="acc", name="dn")
                    ao_p = psACC.tile([128, 512], F32, tag="acc", name="ao")
                    state[(qc, p)] = (ao_p, dn_p)
                ao_p, dn_p = state[(qc, p)]
                # first MM of the k==0 group clears the whole bank;
                # the second must NOT re-clear (would drop the
                # first's has_written bits) -> start only on MM1.
                st = (k == 0)
                sp = (k == KTC - 1)
                nc.tensor.matmul(ao_p[0:64, :], v_sb[:, k, 2 * p, :],
                                 eS[:, 0, :], start=st, stop=sp,
                                 tile_position=(0, 0),
                                 skip_group_check=True)
                nc.tensor.matmul(ao_p[64:128, :], v_sb[:, k, 2 * p + 1, :],
                                 eS[:, 1, :], start=st, stop=sp,
                                 tile_position=(0, 64),
                                 skip_group_check=True)
                # denominator rows 0 and 64: opposite column quadrants so the
                # two M=1 streams run concurrently
                nc.tensor.matmul(dn_p[0:1, :], ones1, eS[:, 0, :],
                                 start=st, stop=sp,
                                 tile_position=(0, 0),
                                 skip_group_check=True)
                nc.tensor.matmul(dn_p[64:65, :], ones1, eS[:, 1, :],
                                 start=st, stop=sp,
                                 tile_position=(0, 64),
                                 skip_group_check=True)

            gst = {}  # (qc, p) -> (sig, dns) from gating_a

            def gating_a(qc, p):
                # DVE/ACT half of gating: runs as soon as dn completes
                _, dn_p = state[(qc, p)]
                gs = gat_p.tile([128, 512], F32, tag="gs")
                nc.scalar.activation(out=gs, in_=graw[qc][:, p, :],
                                     func=AF.Tanh, scale=0.5)
                sig = gat_p.tile([128, 512], F32, tag="sig")
                nc.vector.tensor_scalar(out=sig, in0=gs, scalar1=0.5,
                                        scalar2=0.5, op0=OP.mult,
                                        op1=OP.add)
                dns = gat_p.tile([128, 512], BF, tag="dns")
                nc.vector.tensor_copy(out=dns[0:1, :], in_=dn_p[0:1, :])
                nc.vector.tensor_copy(out=dns[64:65, :], in_=dn_p[64:65, :])
                gst[(qc, p)] = (sig, dns)

            def gating_b(qc, p, rbc=None):
                # PE broadcast of 1/denominator + final gated A: two steps
                # after gating_a so the rbc matmuls never wait on DVE
                ao_p, dn_p = state.pop((qc, p))
                sig, dns = gst.pop((qc, p))
                if rbc is None:
                    rbc = cur_sc["ps"][:, 512:1024]
                nc.tensor.matmul(rbc[0:64, :], ones2[0:1, :], dns[0:1, :],
                                 start=True, stop=True,
                                 tile_position=(0, 0), skip_group_check=True)
                nc.tensor.matmul(rbc[64:128, :], ones2[64:65, :],
                                 dns[64:65, :], start=True, stop=True,
                                 tile_position=(64, 64),
                                 skip_group_check=True)
                rec = gat_p.tile([128, 512], F32, tag="rec")
                nc.vector.reciprocal(out=rec, in_=rbc)
                m = gat_p.tile([128, 512], F32, tag="m")
                nc.vector.tensor_tensor(out=m, in0=sig, in1=rec, op=OP.mult)
                nc.vector.tensor_tensor(out=A_sb[qc][:, p, :], in0=ao_p,
                                        in1=m, op=OP.mult)

            def outproj_piece(qc, i, po=None):
                # accumulates in the just-consumed scores tile's second bank:
                # no pool alloc, so the scores ring and proj bank are untouched
                nk, oc = divmod(i, 2)
                n1 = qc * 4 + nk
                if po is None:
                    po = cur_sc["ps"][:, 512:1024]
                for fc in range(2):
                    nc.tensor.matmul(
                        po,
                        A_sb[qc][:, fc, nk * 128:(nk + 1) * 128],
                        wo_sb[:, fc, oc * 512:(oc + 1) * 512],
                        start=(fc == 0), stop=(fc == 1),
                        skip_group_check=True)
                ev = gat_p.tile([128, 512], F32, tag="ev")
                nc.vector.tensor_copy(out=ev, in_=po)
                nc.sync.dma_start(
                    out=part.ap()[n1 * 128:(n1 + 1) * 128,
                                  oc * 512:(oc + 1) * 512],
                    in_=ev)

            hooks = {}

            def add_hook(qc, p, k, fn):
                hooks.setdefault((qc, p, k), []).append(fn)

            def at(qc, p, k):
                # normalize a possibly-overflowing (qc, p, k) step address
                t = qc * 32 + p * 16 + k
                return t // 32, (t % 32) // 16, t % 16

            def add_stages(first, stages, gap=1):
                """Register stage list at consecutive steps (gap apart); the
                final stage (transpose+evac) runs 3 steps after the prior."""
                qc, p, k = first
                for i, s in enumerate(stages[:-1]):
                    add_hook(*at(qc, p, k + i * gap), s)
                add_hook(*at(qc, p, k + (len(stages) - 2) * gap + 3),
                         stages[-1])

            for qc in range(4):
                # gating as soon as the pair's accumulation completes
                # (leftover flushes drain by k=1)
                add_hook(qc, 1, 2, lambda qc=qc: gating_a(qc, 0))
                add_hook(qc, 1, 4, lambda qc=qc: gating_b(qc, 0))
                if qc > 0:
                    add_hook(qc, 0, 2, lambda qc=qc: gating_a(qc - 1, 1))
                    add_hook(qc, 0, 3, lambda qc=qc: gating_b(qc - 1, 1))
                    for i in range(8):
                        add_hook(qc, 0, 5 + i,
                                 lambda qc=qc, i=i: outproj_piece(qc - 1, i))
                if qc + 1 < QB:
                    # next q-block projections spread through this window
                    qcn = qc + 1
                    b = []
                    add_hook(qc, 0, 1, lambda qcn=qcn, b=b:
                             b.append(load_x_slice(qcn)))

                    def lazy_stages(factory, n):
                        box2 = {}

                        def run(i):
                            if "s" not in box2:
                                box2["s"] = factory()
                            box2["s"][i]()

                        return [lambda i=i: run(i) for i in range(n)]

                    def mk(ns, qcn=qcn, b=b):
                        return lazy_stages(
                            lambda: q_stages(b[0], ns, qcn), 5)

                    add_stages((qc, 0, 11), mk(0))
                    add_stages((qc, 0, 15), mk(1))
                    add_stages((qc, 1, 3), mk(2))
                    add_stages((qc, 1, 7), mk(3))

                    def mkg(gfc, qcn=qcn, b=b):
                        return lazy_stages(
                            lambda: gate_stages(b[0], gfc, qcn), 4)

                    g0 = mkg(0)
                    g1 = mkg(1)
                    for i in range(4):
                        add_hook(*at(qc, 1, 11 + i), g0[i])
                        add_hook(*at(qc, 1, 15 + i), g1[i])

            pend = []  # (expS tile, qc, p, ktc) awaiting attn MMs
            for qc in range(4):
                for p in range(2):
                    for k in range(KTC):
                        # drain the previous pair's leftovers two groups per
                        # step (done by k=2, before gating_a); otherwise keep
                        # a 4-5 step trail so gating_b precedes the psACC
                        # slot rotation it feeds
                        if k <= 2:
                            nf = 0
                            while (pend and pend[0][1:3] != (qc, p)
                                   and nf < 2):
                                flush_attn(*pend.pop(0))
                                nf += 1
                        while len(pend) >= 5:
                            flush_attn(*pend.pop(0))
                        ksl = slice(k * 128, (k + 1) * 128)
                        ps = psSC.tile([128, 1024], F32, tag="sc", name="sc")
                        nc.tensor.matmul(ps[:, 0:512],
                                         pairK[0:64, p, ksl],
                                         pairQb[qc][0:64, p, :],
                                         start=True, stop=True,
                                         tile_position=(0, 0))
                        nc.tensor.matmul(ps[:, 512:1024],
                                         pairK[64:128, p, ksl],
                                         pairQb[qc][64:128, p, :],
                                         start=True, stop=True,
                                         tile_position=(64, 0))
                        eS = exps_p.tile([128, 2, 512], BF, tag="expS",
                                         name="eS")
                        nc.scalar.activation(
                            out=eS.rearrange("p a b -> p (a b)"), in_=ps,
                            func=AF.Exp, scale=0.125)
                        pend.append((eS, qc, p, k))
                        cur_sc["ps"] = ps
                        for fn in hooks.pop((qc, p, k), ()):
                            fn()
            for e in pend:
                flush_attn(*e)
            gating_a(3, 1)
            rbt = psSC.tile([128, 1024], F32, tag="sc", name="rbt")
            gating_b(3, 1, rbc=rbt[:, 0:512])
            # tail out-proj pipelined through fresh scores-pool tiles
            for i in range(0, 8, 2):
                tpo = psSC.tile([128, 1024], F32, tag="sc", name="tpo")
                outproj_piece(3, i, po=tpo[:, 0:512])
                outproj_piece(3, i + 1, po=tpo[:, 512:1024])

    nc.compile()
    return nc


def _prep_core(inputs, b, g, bf16):
    x = np.asarray(inputs["x"][b], dtype=np.float32)
    ctx = np.asarray(inputs["context"][b], dtype=np.float32)
    Wq = np.asarray(inputs["Wq"], dtype=np.float32).reshape(H, 2 * D, C)
    Wkv = np.asarray(inputs["Wkv"], dtype=np.float32).reshape(H, 2 * D, C)
    Wo = np.asarray(inputs["Wo"], dtype=np.float32)
    cos = np.asarray(inputs["cos"][b], dtype=np.float32)
    sin = np.asarray(inputs["sin"][b], dtype=np.float32)
    qw = np.asarray(inputs["q_norm_w"], dtype=np.float32)
    kw = np.asarray(inputs["k_norm_w"], dtype=np.float32)

    hs = slice(HG * g, HG * g + HG)
    qr = Wq[hs, :D, :]                       # [4, D, C]
    qr = qr - qr.mean(axis=1, keepdims=True)
    gr = Wq[hs, D:, :]
    kr = Wkv[hs, :D, :]
    kr = kr - kr.mean(axis=1, keepdims=True)
    vr = Wkv[hs, D:, :]

    sgn = np.where(np.arange(D) < D // 2, -1.0, 1.0).astype(np.float32)
    wswap = lambda w: np.concatenate([w[D // 2:], w[:D // 2]])

    def pmajor(a, nch, f):
        # [C, F] c-major -> [128, nch, F] partition-major contiguous
        return np.ascontiguousarray(
            a.reshape(nch, 128, f).transpose(1, 0, 2)).astype(bf16)

    def tab(a):
        # [N, D] -> [128, NCH, D] token-chunked partition-major
        return np.ascontiguousarray(
            a.reshape(NCH, 128, D).transpose(1, 0, 2)).astype(bf16)

    return {
        "xT": np.ascontiguousarray(x.T).reshape(8, 128, N).astype(bf16),
        "ctxT": np.ascontiguousarray(ctx.T).reshape(8, 128, N).astype(bf16),
        "wq": pmajor(qr.reshape(HG * D, C).T, 8, 256),
        "wg": pmajor(gr.reshape(HG * D, C).T, 8, 256),
        "wkv": pmajor(np.concatenate(
            [kr.reshape(HG * D, C), vr.reshape(HG * D, C)], 0).T, 8, 512),
        "wo": pmajor(Wo[:, 256 * g:256 * (g + 1)].T, 2, C),
        "cosq": tab(cos * qw[None, :]),
        "ssinq": tab(sin * sgn[None, :] * wswap(qw)[None, :]),
        "cosk": tab(cos * kw[None, :]),
        "ssink": tab(sin * sgn[None, :] * wswap(kw)[None, :]),
    }


def kernel(**inputs):
    global _PROG, LAST_EXEC_NS, LAST_PROFILE
    import ml_dtypes
    bf16 = ml_dtypes.bfloat16

    if _PROG is None:
        _PROG = _build_program()
    nc = _PROG

    in_maps = [_prep_core(inputs, core // 4, core % 4, bf16) for core in range(8)]

    trace = bool(os.environ.get("BASS_KERNEL_TRACE"))
    kw = {}
    if trace:
        import types
        from trn_agent_boot.trn_boot import _ntff_profile_via_ctypes
        hook = _ntff_profile_via_ctypes('/opt/axon/libaxon_pjrt.so')
        mod = types.ModuleType('antenv.axon_hooks')
        mod.get_axon_ntff_profile_hook = lambda: hook
        sys.modules['antenv.axon_hooks'] = mod
        from concourse import bass_utils
        bass_utils.upload_artifacts = lambda tmpdir: tmpdir
        kw = dict(trace=True, tmpdir=os.environ.get("BASS_KERNEL_TRACE_DIR"))

    from concourse.bass_utils import run_bass_kernel_spmd
    res = run_bass_kernel_spmd(nc, in_maps, core_ids=list(range(8)), **kw)
    LAST_EXEC_NS = res.exec_time_ns
    LAST_PROFILE = res.profile_json

    bo = np.asarray(inputs["bo"], dtype=np.float32)
    out = np.zeros((B, N, C), dtype=np.float32)
    for core in range(8):
        out[core // 4] += res.results[core]["part"]
    out += bo[None, None, :]
    return out
